# revision 3
# baseline (speedup 1.0000x reference)
"""TRN2 kernel: segmented histogram / graph label counting (nn_Counting_Encoding).

kernel(**inputs) takes FULL (unsharded) inputs and returns the FULL
[num_graphs, out_dim] float32 output:

    out[g, c] = sum over nodes i in graph g of node_weights[i] * (x[i] == c)

Strategy
--------
Host: derive per-node graph ids from CSR `ptr`, size-sort graphs and deal them
round-robin to the 8 NeuronCores so every core runs an IDENTICAL (SPMD) chunk
schedule; expand each node into two tiny fp8 one-hots (hi = x>>4, lo = x&15,
16-wide each; weight folded into hi) packed per-graph into 128-node chunks.

Device (per core): for each scheduled graph "slot", accumulate
    hist[hi, lo] += HI_chunk.T @ LO_chunk
on the PE array — fp8 DoubleRow matmuls contract 256 nodes each (plain matmul
for odd tails); a slot's chunks accumulate into a PSUM [16,16] region; 16 slots
share a PSUM bank; PSUM -> SBUF -> DMA out. Input streaming and output DMAs use
different engine queues so block prefetch overlaps compute.

Host: gather the 8 per-core [slots, 256] buffers and scatter rows back to
graph order. All arithmetic is exact (0/1 products, fp32 accumulation).
"""
import os

import numpy as np
import ml_dtypes

import concourse.bass as bass
import concourse.tile as tile
from concourse import bacc, mybir
from concourse.bass_utils import run_bass_kernel_spmd

N_CORES = 8
K = 128          # partitions = contraction rows per half-chunk
CHUNK = 256      # nodes per DoubleRow matmul
H = 16           # hi one-hot width
L = 16           # lo one-hot width
CW = 2 * H       # fp8 columns per DoubleRow matmul operand
GG = 16          # slots per PSUM group
GROUPS_PER_BLOCK = 8
SLOTS_PER_BLOCK = GG * GROUPS_PER_BLOCK   # 128
FP8 = mybir.dt.float8e4
FP8_NP = ml_dtypes.float8_e4m3
DR = mybir.MatmulPerfMode.DoubleRow
USE_DR = os.environ.get("USE_DR", "1") == "1"

LAST_RESULTS = {}
_BUILD_CACHE = {}


def _make_schedule(counts):
    """Per-slot 128-node chunk counts (identical across cores) + slot->graph map."""
    C = -(-counts // K)
    order = np.argsort(-counts, kind="stable")
    order = order[counts[order] > 0]
    n = order.shape[0]
    npos = -(-n // N_CORES)
    S = -(-npos // SLOTS_PER_BLOCK) * SLOTS_PER_BLOCK
    n_blocks = S // SLOTS_PER_BLOCK

    pos_C = np.ones(S, dtype=np.int64)
    pos_C[:npos] = np.maximum(1, C[order[::N_CORES][:npos]])
    pos_graph = np.full((N_CORES, S), -1, dtype=np.int64)
    for m in range(N_CORES):
        g = order[m::N_CORES]
        pos_graph[m, : g.shape[0]] = g

    # stripe slots across blocks so each block gets a near-equal chunk count
    perm = np.arange(S).reshape(-1, n_blocks).T.reshape(-1)
    return pos_C[perm], pos_graph[:, perm]


def _build_bass(sched_C):
    """Build + compile the SPMD bass program for the given chunk schedule."""
    S = len(sched_C)
    assert S % SLOTS_PER_BLOCK == 0
    n_blocks = S // SLOTS_PER_BLOCK
    off = np.zeros(S + 1, dtype=np.int64)
    np.cumsum(sched_C, out=off[1:])
    T = int(off[-1])

    nc = bacc.Bacc("TRN2", target_bir_lowering=False, debug=False)
    hh = nc.dram_tensor("hh", [K, T * H], FP8, kind="ExternalInput")
    lh = nc.dram_tensor("lh", [K, T * H], FP8, kind="ExternalInput")
    out = nc.dram_tensor("out", [S * 256], mybir.dt.float32, kind="ExternalOutput")

    with tile.TileContext(nc) as tc:
        with (
            tc.tile_pool(name="in_hh", bufs=3) as hh_pool,
            tc.tile_pool(name="in_lh", bufs=3) as lh_pool,
            tc.tile_pool(name="psum", bufs=8, space=bass.MemorySpace.PSUM) as psum_pool,
            tc.tile_pool(name="outs", bufs=2) as out_pool,
        ):
            for b in range(n_blocks):
                j0 = b * SLOTS_PER_BLOCK
                c0 = int(off[j0])
                cols = (int(off[j0 + SLOTS_PER_BLOCK]) - c0) * H
                hh_t = hh_pool.tile([K, cols], FP8, tag="hh")
                lh_t = lh_pool.tile([K, cols], FP8, tag="lh")
                # inputs on the sync queue, outputs on the gpsimd queue: keeps
                # block b+1's input prefetch from serializing behind block b's
                # output DMA (engine queues are in-order)
                nc.sync.dma_start(hh_t[:], hh[:, c0 * H: c0 * H + cols])
                nc.sync.dma_start(lh_t[:], lh[:, c0 * H: c0 * H + cols])
                out_t = out_pool.tile(
                    [H, SLOTS_PER_BLOCK * L], mybir.dt.float32, tag="out"
                )
                for grp in range(GROUPS_PER_BLOCK):
                    ps = psum_pool.tile([H, GG * L], mybir.dt.float32, tag="ps")
                    for s in range(GG):
                        j = j0 + grp * GG + s
                        nch = int(sched_C[j])
                        lo0 = int(off[j]) - c0
                        c = 0
                        first = True
                        while c < nch:
                            colo = (lo0 + c) * H
                            if USE_DR and c + 1 < nch:
                                nc.tensor.matmul(
                                    ps[:, s * L:(s + 1) * L],
                                    hh_t[:, colo: colo + CW].rearrange(
                                        "p (two h) -> p two h", two=2
                                    ),
                                    lh_t[:, colo: colo + CW].rearrange(
                                        "p (two h) -> p two h", two=2
                                    ),
                                    start=first,
                                    stop=(c + 2 >= nch),
                                    perf_mode=DR,
                                )
                                c += 2
                            else:
                                nc.tensor.matmul(
                                    ps[:, s * L:(s + 1) * L],
                                    hh_t[:, colo: colo + H],
                                    lh_t[:, colo: colo + H],
                                    start=first,
                                    stop=(c + 1 >= nch),
                                )
                                c += 1
                            first = False
                    nc.vector.tensor_copy(
                        out_t[:, grp * GG * L:(grp + 1) * GG * L], ps[:]
                    )
                dram_ap = bass.AP(
                    out, j0 * 256, [[L, H], [256, SLOTS_PER_BLOCK], [1, L]]
                )
                nc.gpsimd.dma_start(dram_ap, out_t[:])
    nc.compile()
    return nc


def _host_prep(x, ptr, node_weights, num_graphs, out_dim):
    """Shard + expand nodes into per-core fp8 one-hot arrays."""
    N = x.shape[0]
    x = np.asarray(x, dtype=np.int64)
    ptr = np.asarray(ptr, dtype=np.int64)
    w = np.asarray(node_weights, dtype=np.float32)

    counts = np.diff(ptr)
    sched_C, slot_graph = _make_schedule(counts)
    S = len(sched_C)
    off = np.zeros(S + 1, dtype=np.int64)
    np.cumsum(sched_C, out=off[1:])
    T = int(off[-1])

    core_of = np.full(num_graphs, -1, dtype=np.int64)
    slot_of = np.full(num_graphs, -1, dtype=np.int64)
    for m in range(N_CORES):
        valid = slot_graph[m] >= 0
        core_of[slot_graph[m][valid]] = m
        slot_of[slot_graph[m][valid]] = np.nonzero(valid)[0]

    seg = np.repeat(np.arange(num_graphs, dtype=np.int64), counts)
    pos = np.arange(N, dtype=np.int64) - ptr[seg]

    core = core_of[seg]
    slot = slot_of[seg]
    ki = pos % K
    ch = pos // K
    hi = x >> 4
    lo = x & 15
    valid = x < out_dim

    cols = T * H
    hh = np.zeros((N_CORES, K, cols), dtype=FP8_NP)
    lh = np.zeros((N_CORES, K, cols), dtype=FP8_NP)
    colbase = (off[slot] + ch) * H
    flat = (core * K + ki) * cols + colbase
    hh.reshape(-1)[(flat + hi)[valid]] = w[valid].astype(FP8_NP)
    lh.reshape(-1)[(flat + lo)[valid]] = np.float32(1.0)

    in_maps = [{"hh": hh[m], "lh": lh[m]} for m in range(N_CORES)]
    return in_maps, sched_C, slot_graph


def kernel(x, ptr, node_weights, num_graphs, out_dim):
    num_graphs = int(num_graphs)
    out_dim = int(out_dim)
    x = np.asarray(x)
    ptr = np.asarray(ptr)
    node_weights = np.asarray(node_weights)

    trace = os.environ.get("TRACE", "0") == "1"
    if trace:
        try:
            import ntff_shim

            ntff_shim.install()
        except ImportError:
            pass

    in_maps, sched_C, slot_graph = _host_prep(
        x, ptr, node_weights, num_graphs, out_dim
    )
    key = sched_C.tobytes()
    nc = _BUILD_CACHE.get(key)
    if nc is None:
        nc = _build_bass(sched_C)
        _BUILD_CACHE[key] = nc
    res = run_bass_kernel_spmd(
        nc, in_maps, core_ids=list(range(N_CORES)), trace=trace
    )
    LAST_RESULTS["res"] = res

    full = np.zeros((num_graphs, 256), dtype=np.float32)
    S = slot_graph.shape[1]
    for m in range(N_CORES):
        rows = res.results[m]["out"].reshape(S, 256)
        valid = slot_graph[m] >= 0
        full[slot_graph[m][valid]] = rows[valid]
    return full[:, :out_dim]


# revision 7
# speedup vs baseline: 1.3198x; 1.3198x over previous
"""TRN2 kernel: segmented histogram / graph label counting (nn_Counting_Encoding).

kernel(**inputs) takes FULL (unsharded) inputs and returns the FULL
[num_graphs, out_dim] float32 output:

    out[g, c] = sum over nodes i in graph g of node_weights[i] * (x[i] == c)

Strategy
--------
Host: derive per-node graph ids from CSR `ptr`, size-sort graphs and deal them
round-robin to the 8 NeuronCores so every core runs an IDENTICAL (SPMD) chunk
schedule; expand each node into two tiny fp8 one-hots (hi = x>>4, lo = x&15,
16-wide each; weight folded into hi) packed per-graph into 128-node chunks.

Device (per core): for each scheduled graph "slot", accumulate
    hist[hi, lo] += HI_chunk.T @ LO_chunk
on the PE array — fp8 DoubleRow matmuls contract 256 nodes each (plain matmul
for odd tails); a slot's chunks accumulate into a PSUM [16,16] region; 16 slots
share a PSUM bank; PSUM -> SBUF -> DMA out. Input streaming and output DMAs use
different engine queues so block prefetch overlaps compute.

Host: gather the 8 per-core [slots, 256] buffers and scatter rows back to
graph order. All arithmetic is exact (0/1 products, fp32 accumulation).
"""
import os

import numpy as np
import ml_dtypes

import concourse.bass as bass
import concourse.tile as tile
from concourse import bacc, mybir
from concourse.bass_utils import run_bass_kernel_spmd

N_CORES = 8
K = 128          # partitions = contraction rows per half-chunk
CHUNK = 256      # nodes per DoubleRow matmul
H = 16           # hi one-hot width
L = 16           # lo one-hot width
CW = 2 * H       # fp8 columns per DoubleRow matmul operand
GG = 16          # slots per PSUM group
GROUPS_PER_BLOCK = 8
SLOTS_PER_BLOCK = GG * GROUPS_PER_BLOCK   # 128
FP8 = mybir.dt.float8e4
FP8_NP = ml_dtypes.float8_e4m3
DR = mybir.MatmulPerfMode.DoubleRow
USE_DR = os.environ.get("USE_DR", "1") == "1"

LAST_RESULTS = {}
_BUILD_CACHE = {}


def _make_schedule(counts):
    """Per-slot 128-node chunk counts (identical across cores) + slot->graph map."""
    C = -(-counts // K)
    order = np.argsort(-counts, kind="stable")
    order = order[counts[order] > 0]
    n = order.shape[0]
    npos = -(-n // N_CORES)
    S = -(-npos // SLOTS_PER_BLOCK) * SLOTS_PER_BLOCK
    n_blocks = S // SLOTS_PER_BLOCK

    pos_C = np.ones(S, dtype=np.int64)
    pos_C[:npos] = np.maximum(1, C[order[::N_CORES][:npos]])
    pos_graph = np.full((N_CORES, S), -1, dtype=np.int64)
    for m in range(N_CORES):
        g = order[m::N_CORES]
        pos_graph[m, : g.shape[0]] = g

    # stripe slots across blocks so each block gets a near-equal chunk count
    perm = np.arange(S).reshape(-1, n_blocks).T.reshape(-1)
    return pos_C[perm], pos_graph[:, perm]


def _build_bass(sched_C, use_dr=True):
    """Build + compile the SPMD bass program for the given chunk schedule."""
    S = len(sched_C)
    assert S % SLOTS_PER_BLOCK == 0
    n_blocks = S // SLOTS_PER_BLOCK
    pairs = (np.asarray(sched_C) + 1) // 2
    off = np.zeros(S + 1, dtype=np.int64)
    np.cumsum(pairs, out=off[1:])         # 256-node (32B-aligned) pair storage
    T = int(off[-1])

    nc = bacc.Bacc("TRN2", target_bir_lowering=False, debug=False)
    hh = nc.dram_tensor("hh", [K, T * CW], FP8, kind="ExternalInput")
    lh = nc.dram_tensor("lh", [K, T * CW], FP8, kind="ExternalInput")
    out = nc.dram_tensor("out", [S * 256], mybir.dt.float32, kind="ExternalOutput")

    with tile.TileContext(nc) as tc:
        with (
            tc.tile_pool(name="in_hh", bufs=3) as hh_pool,
            tc.tile_pool(name="in_lh", bufs=3) as lh_pool,
            tc.tile_pool(name="psum", bufs=8, space=bass.MemorySpace.PSUM) as psum_pool,
            tc.tile_pool(name="outs", bufs=2) as out_pool,
        ):
            for b in range(n_blocks):
                j0 = b * SLOTS_PER_BLOCK
                c0 = int(off[j0])
                cols = (int(off[j0 + SLOTS_PER_BLOCK]) - c0) * CW
                hh_t = hh_pool.tile([K, cols], FP8, tag="hh")
                lh_t = lh_pool.tile([K, cols], FP8, tag="lh")
                # inputs on the sync queue, outputs on the gpsimd queue: keeps
                # block b+1's input prefetch from serializing behind block b's
                # output DMA (engine queues are in-order)
                nc.sync.dma_start(hh_t[:], hh[:, c0 * CW: c0 * CW + cols])
                nc.sync.dma_start(lh_t[:], lh[:, c0 * CW: c0 * CW + cols])
                out_t = out_pool.tile(
                    [H, SLOTS_PER_BLOCK * L], mybir.dt.float32, tag="out"
                )
                for grp in range(GROUPS_PER_BLOCK):
                    ps = psum_pool.tile([H, GG * L], mybir.dt.float32, tag="ps")
                    for s in range(GG):
                        j = j0 + grp * GG + s
                        npair = (int(sched_C[j]) + 1) // 2
                        lo0 = int(off[j]) - c0
                        for c in range(npair):
                            colo = (lo0 + c) * CW
                            first = c == 0
                            if use_dr:
                                nc.tensor.matmul(
                                    ps[:, s * L:(s + 1) * L],
                                    hh_t[:, colo: colo + CW].rearrange(
                                        "p (two h) -> p two h", two=2
                                    ),
                                    lh_t[:, colo: colo + CW].rearrange(
                                        "p (two h) -> p two h", two=2
                                    ),
                                    start=first,
                                    stop=(c == npair - 1),
                                    perf_mode=DR,
                                )
                            else:
                                for hf in range(2):
                                    nc.tensor.matmul(
                                        ps[:, s * L:(s + 1) * L],
                                        hh_t[:, colo + hf * H: colo + (hf + 1) * H],
                                        lh_t[:, colo + hf * H: colo + (hf + 1) * H],
                                        start=(first and hf == 0),
                                        stop=(c == npair - 1 and hf == 1),
                                    )
                    nc.vector.tensor_copy(
                        out_t[:, grp * GG * L:(grp + 1) * GG * L], ps[:]
                    )
                dram_ap = bass.AP(
                    out, j0 * 256, [[L, H], [256, SLOTS_PER_BLOCK], [1, L]]
                )
                nc.gpsimd.dma_start(dram_ap, out_t[:])
    nc.compile()
    return nc


def _host_prep(x, ptr, node_weights, num_graphs, out_dim):
    """Shard + expand nodes into per-core fp8 one-hot arrays."""
    N = x.shape[0]
    x = np.asarray(x, dtype=np.int64)
    ptr = np.asarray(ptr, dtype=np.int64)
    w = np.asarray(node_weights, dtype=np.float32)

    counts = np.diff(ptr)
    sched_C, slot_graph = _make_schedule(counts)
    S = len(sched_C)
    pairs = (sched_C + 1) // 2
    off = np.zeros(S + 1, dtype=np.int64)
    np.cumsum(pairs, out=off[1:])
    T = int(off[-1])

    core_of = np.full(num_graphs, -1, dtype=np.int64)
    slot_of = np.full(num_graphs, -1, dtype=np.int64)
    for m in range(N_CORES):
        valid = slot_graph[m] >= 0
        core_of[slot_graph[m][valid]] = m
        slot_of[slot_graph[m][valid]] = np.nonzero(valid)[0]

    seg = np.repeat(np.arange(num_graphs, dtype=np.int64), counts)
    pos = np.arange(N, dtype=np.int64) - ptr[seg]

    core = core_of[seg]
    slot = slot_of[seg]
    ki = pos % K
    half = (pos % CHUNK) // K
    pair = pos // CHUNK
    hi = x >> 4
    lo = x & 15
    valid = x < out_dim

    cols = T * CW
    hh = np.zeros((N_CORES, K, cols), dtype=FP8_NP)
    lh = np.zeros((N_CORES, K, cols), dtype=FP8_NP)
    colbase = (off[slot] + pair) * CW + half * H
    flat = (core * K + ki) * cols + colbase
    hh.reshape(-1)[(flat + hi)[valid]] = w[valid].astype(FP8_NP)
    lh.reshape(-1)[(flat + lo)[valid]] = np.float32(1.0)

    in_maps = [{"hh": hh[m], "lh": lh[m]} for m in range(N_CORES)]
    return in_maps, sched_C, slot_graph


def kernel(x, ptr, node_weights, num_graphs, out_dim):
    num_graphs = int(num_graphs)
    out_dim = int(out_dim)
    x = np.asarray(x)
    ptr = np.asarray(ptr)
    node_weights = np.asarray(node_weights)

    trace = os.environ.get("TRACE", "0") == "1"
    if trace:
        try:
            import ntff_shim

            ntff_shim.install()
        except ImportError:
            pass

    in_maps, sched_C, slot_graph = _host_prep(
        x, ptr, node_weights, num_graphs, out_dim
    )
    core_ids = list(range(N_CORES))
    res = None
    attempts = ([True, False] if USE_DR else [False])
    last_exc = None
    for use_dr in attempts:
        key = (sched_C.tobytes(), use_dr)
        nc = _BUILD_CACHE.get(key)
        if nc is None:
            nc = _build_bass(sched_C, use_dr=use_dr)
            _BUILD_CACHE[key] = nc
        try:
            res = run_bass_kernel_spmd(nc, in_maps, core_ids=core_ids, trace=trace)
            break
        except Exception as e:  # device-level failure: retry without DoubleRow
            last_exc = e
            continue
    if res is None:
        raise last_exc
    LAST_RESULTS["res"] = res

    full = np.zeros((num_graphs, 256), dtype=np.float32)
    S = slot_graph.shape[1]
    for m in range(N_CORES):
        rows = res.results[m]["out"].reshape(S, 256)
        valid = slot_graph[m] >= 0
        full[slot_graph[m][valid]] = rows[valid]
    return full[:, :out_dim]
